# revision 33
# baseline (speedup 1.0000x reference)
"""Trainium2 Bass kernel for nn_Current_44435731645048.

Model: per-channel bidirectional temporal RNN gating (H=256, seq 50) over
4096 scalar sequences, followed by a 2-layer GRU (H=256, T=128, batch 16).

Sharding: temporal phase is data-parallel over the 4096 sequences (512/core,
grouped so the AllGather output is directly the GRU's time-major input
layout); the GRU phase is replicated on every core (batch 16 is too small to
shard profitably; the scan is latency-bound).

Self-contained: hardcodes all shapes; only needs /opt/trn_rl_repo.
"""
import os
import sys

sys.path.insert(0, '/opt/trn_rl_repo')

import numpy as np

import concourse.bass as bass
import concourse.bacc as bacc
import concourse.mybir as mybir
import concourse.tile as tile
from concourse.bass_utils import run_bass_kernel_spmd

F32 = mybir.dt.float32
F32R = mybir.dt.float32r
F16 = mybir.dt.float16
AF = mybir.ActivationFunctionType
ALU = mybir.AluOpType

H = 256
M = 50        # temporal sequence length
B = 16        # GRU batch
Z = 128       # GRU sequence length
NC = 8        # cores
S = 512       # temporal sequences per core

_last_results = None
_nc_cache = {}


def _r(ap):
    """bitcast an fp32 AP to float32r for full-rate PE matmul."""
    return ap.bitcast(F32R)


def build_nc(sim_mode=False):
    nc = bacc.Bacc("TRN2", target_bir_lowering=False, debug=False,
                   num_devices=1 if sim_mode else NC)

    # ---- per-core inputs ----
    d_xs100 = nc.dram_tensor("xs100", [2 * M, S], F16, kind="ExternalInput")
    d_xseqT = nc.dram_tensor("xseqT", [M, S], F32, kind="ExternalInput")
    # ---- replicated weights ----
    d_whhT_f = nc.dram_tensor("whhT_f", [128, 512], F16, kind="ExternalInput")
    d_whhT_b = nc.dram_tensor("whhT_b", [128, 512], F16, kind="ExternalInput")
    d_wxb_f = nc.dram_tensor("wxb_f", [2, 256], F16, kind="ExternalInput")
    d_wxb_b = nc.dram_tensor("wxb_b", [2, 256], F16, kind="ExternalInput")
    d_fusW = nc.dram_tensor("fusW", [128, 4], F16, kind="ExternalInput")
    d_fusb = nc.dram_tensor("fusb", [128, 1], F32, kind="ExternalInput")
    d_wih0T = nc.dram_tensor("wih0T", [100, 768], F16, kind="ExternalInput")
    d_whh0T = nc.dram_tensor("whh0T", [128, 1536], F16, kind="ExternalInput")
    d_whh1T = nc.dram_tensor("whh1T", [128, 1536], F16, kind="ExternalInput")
    d_wih1T = nc.dram_tensor("wih1T", [128, 1536], F16, kind="ExternalInput")
    d_bhn0 = nc.dram_tensor("bhn0b", [128, 32], F16, kind="ExternalInput")
    d_bhn1 = nc.dram_tensor("bhn1b", [128, 32], F16, kind="ExternalInput")
    d_ident = nc.dram_tensor("ident", [128, 128], F16, kind="ExternalInput")
    d_scal = nc.dram_tensor("scal", [128, 12], F32, kind="ExternalInput")
    d_h0p = nc.dram_tensor("h0p", [128, 64], F32, kind="ExternalInput")
    # ---- outputs ----
    d_out1 = nc.dram_tensor("out1", [128, Z * 32], F16, kind="ExternalOutput")
    d_hn = nc.dram_tensor("hn", [128, 64], F16, kind="ExternalOutput")
    taps = os.environ.get("BASS_DEBUG_TAPS", "0") == "1"
    if taps:
        d_dbgG = nc.dram_tensor("dbgG", [M, S], F32, kind="ExternalOutput")
        d_dbgxg = nc.dram_tensor("dbgxg", [100, Z * B], F32, kind="ExternalOutput")
        d_dbgrz = nc.dram_tensor("dbgrz", [128, Z * 64], F16, kind="ExternalOutput")
        d_dbgn = nc.dram_tensor("dbgn", [128, Z * 32], F32, kind="ExternalOutput")
        d_dbgout0 = nc.dram_tensor("dbgout0", [128, Z * 32], F16, kind="ExternalOutput")

    with tile.TileContext(nc) as tc:
        with (
            tc.tile_pool(name="const", bufs=1) as cp,
            tc.tile_pool(name="dram", bufs=1, space="DRAM") as dp,
        ):
            # ------- load constants to SBUF -------
            def load(dram_t, shape, dtype, tag):
                t = cp.tile(shape, dtype, tag=tag)
                nc.sync.dma_start(t[:], dram_t[:])
                return t

            xseqT = load(d_xseqT, [M, S], F32, "xseqT")
            whhT = {0: load(d_whhT_f, [128, 512], F16, "whhTf"),
                    1: load(d_whhT_b, [128, 512], F16, "whhTb")}
            wxb = {0: load(d_wxb_f, [2, 256], F16, "wxbf"),
                   1: load(d_wxb_b, [2, 256], F16, "wxbb")}
            fusW = load(d_fusW, [128, 4], F16, "fusW")
            fusb = load(d_fusb, [128, 1], F32, "fusb")
            wih0T = load(d_wih0T, [100, 768], F16, "wih0T")
            whh0T = load(d_whh0T, [128, 1536], F16, "whh0T")
            whh1T = load(d_whh1T, [128, 1536], F16, "whh1T")
            wih1T = load(d_wih1T, [128, 1536], F16, "wih1T")
            bhn0_b = load(d_bhn0, [128, 32], F16, "bhn0b")
            bhn1_b = load(d_bhn1, [128, 32], F16, "bhn1b")
            ident = load(d_ident, [128, 128], F16, "ident")
            scal = load(d_scal, [128, 12], F32, "scal")
            h0p = load(d_h0p, [128, 64], F32, "h0p")

            # persistent big buffers
            Pf = cp.tile([M, S], F32, tag="Pf")
            Pb = cp.tile([M, S], F32, tag="Pb")
            gi0_rz = cp.tile([128, Z * 64], F16, tag="gi0_rz")
            gi0_n = cp.tile([128, Z * 32], F32, tag="gi0_n")
            out0_16 = cp.tile([128, Z * 32], F16, tag="out0_16")
            out1_16 = cp.tile([128, Z * 32], F16, tag="out1_16")
            xgT32 = cp.tile([100, Z * B], F32, tag="xgT32")
            xgT16 = cp.tile([100, Z * B], F16, tag="xgT16")
            h0_16 = cp.tile([128, 64], F16, tag="h0_16")
            nc.vector.tensor_copy(h0_16[:], h0p[:])

            # ============ PHASE T: temporal bidirectional RNN ============
            with (
                tc.tile_pool(name="hbuf", bufs=3) as hp,
                tc.tile_pool(name="tpsum", bufs=1, space="PSUM") as pp_h,
                tc.tile_pool(name="gpsum", bufs=2, space="PSUM") as pp_g,
                tc.tile_pool(name="twork", bufs=2) as twp,
                tc.tile_pool(name="xrow", bufs=4) as xrp,
            ):
                hprev = {0: None, 1: None}
                for k in range(M):
                    for d in (0, 1):
                        t = k if d == 0 else M - 1 - k
                        xr = xrp.tile([2, S], F16, tag="xr")
                        nc.sync.dma_start(xr[:], d_xs100[2 * t: 2 * t + 2, :])
                        ps = pp_h.tile([128, 1024], F32, tag=f"ps{d}")
                        for m in (0, 1):
                            if k > 0:
                                for kk in (0, 1):
                                    nc.tensor.matmul(
                                        ps[:, m * 512:(m + 1) * 512],
                                        whhT[d][:, kk * 256 + m * 128: kk * 256 + (m + 1) * 128],
                                        hprev[d][:, kk * 512:(kk + 1) * 512],
                                        start=(kk == 0), stop=False,
                                        skip_group_check=True,
                                    )
                            nc.tensor.matmul(
                                ps[:, m * 512:(m + 1) * 512],
                                wxb[d][:, m * 128:(m + 1) * 128],
                                xr[:],
                                start=(k == 0), stop=True,
                                skip_group_check=True,
                            )
                        hnew = hp.tile([128, 1024], F16, tag=f"h{d}")
                        # per-half tanh: frees each PSUM bank as soon as its
                        # 3 matmuls land, so next step's PE work isn't gated
                        # on the whole-tile activation
                        nc.scalar.activation(hnew[:, 0:512], ps[:, 0:512], AF.Tanh)
                        nc.scalar.activation(hnew[:, 512:1024], ps[:, 512:1024], AF.Tanh)
                        # gate partial: p[t] = fW_d . h  -> (1, 512)
                        gp = pp_g.tile([1, 512], F32, tag="gp")
                        for kk in (0, 1):
                            nc.tensor.matmul(
                                gp[:],
                                fusW[:, kk * 2 + d: kk * 2 + d + 1],
                                hnew[:, kk * 512:(kk + 1) * 512],
                                start=(kk == 0), stop=(kk == 1),
                            )
                        P = Pf if d == 0 else Pb
                        gsb = xrp.tile([1, S], F32, tag="gsb")
                        nc.vector.tensor_copy(gsb[:], gp[:])
                        nc.sync.dma_start(P[t:t + 1, :], gsb[:])
                        hprev[d] = hnew

                # gate + output chunk
                logit = twp.tile([M, S], F32, tag="logit")
                nc.vector.tensor_add(logit[:], Pf[:], Pb[:])
                gate = twp.tile([M, S], F32, tag="gate")
                nc.scalar.activation(gate[:], logit[:], AF.Sigmoid,
                                     bias=fusb[0:M, 0:1])
                G = twp.tile([M, S], F32, tag="G")
                nc.vector.tensor_mul(G[:], gate[:], xseqT[:])

                # ============ gather ============
                gchunk = dp.tile([2 * M, 256], F32, tag="gchunk")
                gall = dp.tile([NC * 2 * M, 256], F32, tag="gall")
                nc.sync.dma_start(gchunk[0:M, :], G[:, 0:256])
                nc.sync.dma_start(gchunk[M:2 * M, :], G[:, 256:512])
                if sim_mode:
                    for ii in range(NC):
                        nc.sync.dma_start(gall[ii * 2 * M:(ii + 1) * 2 * M, :], gchunk[:])
                else:
                    nc.gpsimd.collective_compute(
                        "AllGather",
                        ALU.bypass,
                        replica_groups=[list(range(NC))],
                        ins=[gchunk[:].opt()],
                        outs=[gall[:].opt()],
                    )
                nc.sync.dma_start(
                    xgT32[0:100, :].rearrange("p (i c) -> p i c", i=NC),
                    gall[:].rearrange("(i f) c -> f i c", i=NC),
                )
            nc.vector.tensor_copy(xgT16[:], xgT32[:])
            if taps:
                nc.sync.dma_start(d_dbgG[:], G[:])
                nc.sync.dma_start(d_dbgxg[:], xgT32[:])

            _phases = os.environ.get("BASS_PHASES", "full")
            # ============ PHASE G bulk: gi0 = Wih0_aug @ xgT ============
            with tc.tile_pool(name="bpsum", bufs=3, space="PSUM") as bp:
                for ci in range(4 if _phases != "t" else 0):  # column chunks of 512 = 32 timesteps
                    for m in range(6):
                        ps = bp.tile([128, 512], F32, tag="bps")
                        nc.tensor.matmul(
                            ps[:],
                            wih0T[:, m * 128:(m + 1) * 128],
                            xgT16[:, ci * 512:(ci + 1) * 512],
                            start=True, stop=True,
                        )
                        if m < 4:
                            dst = gi0_rz[:].rearrange("p (t c) -> p t c", c=64)[
                                :, ci * 32:(ci + 1) * 32, m * 16:(m + 1) * 16]
                            nc.vector.tensor_scalar_add(
                                dst, ps[:].rearrange("p (t b) -> p t b", b=16),
                                scal[:, 6 + m: 7 + m])
                        else:
                            half = m - 4
                            dst = gi0_n[:].rearrange("p (t c) -> p t c", c=32)[
                                :, ci * 32:(ci + 1) * 32, half * 16:(half + 1) * 16]
                            nc.vector.tensor_scalar_add(
                                dst, ps[:].rearrange("p (t b) -> p t b", b=16),
                                scal[:, 10 + half: 11 + half])

            # ============ PHASE G scan ============
            # Slot s: layer0 step s; every 8 slots a bulk chunk of layer-1
            # input projections (amortizes Wih1 weight loads); layer1 step
            # s-9 (lags one chunk). Bias adds ride on identity matmuls;
            # h'-composition runs on the otherwise-idle GPSIMD engine.
            gi1_rz = cp.tile([128, Z * 64], F16, tag="gi1_rz")
            gi1_n = cp.tile([128, Z * 32], F32, tag="gi1_n")
            with (
                tc.tile_pool(name="r0p", bufs=1, space="PSUM") as pp_r0,
                tc.tile_pool(name="z0p", bufs=1, space="PSUM") as pp_z0,
                tc.tile_pool(name="n0p", bufs=1, space="PSUM") as pp_n0,
                tc.tile_pool(name="r1p", bufs=1, space="PSUM") as pp_r1,
                tc.tile_pool(name="z1p", bufs=1, space="PSUM") as pp_z1,
                tc.tile_pool(name="n1p", bufs=1, space="PSUM") as pp_n1,
                tc.tile_pool(name="ckp", bufs=2, space="PSUM") as pp_ck,
                tc.tile_pool(name="gwork", bufs=6) as gw,
            ):
                def gi1_chunk(c):
                    # gi1[:, 8c:8c+8 steps] = Wih1 @ out0 (+ biases via copies)
                    for m in range(6):
                        ps = pp_ck.tile([128, 128], F32, tag="ck")
                        for kk in (0, 1):
                            rhs = out0_16[:].rearrange(
                                "p (t c) -> p t c", c=32)[
                                :, 8 * c:8 * (c + 1), kk * 16:(kk + 1) * 16]
                            nc.tensor.matmul(
                                ps[:].rearrange("p (t b) -> p t b", b=16),
                                wih1T[:, kk * 768 + m * 128: kk * 768 + (m + 1) * 128],
                                rhs,
                                start=(kk == 0), stop=(kk == 1),
                                skip_group_check=True,
                            )
                        if m < 4:
                            dst = gi1_rz[:].rearrange("p (t c) -> p t c", c=64)[
                                :, 8 * c:8 * (c + 1), m * 16:(m + 1) * 16]
                            nc.vector.tensor_scalar_add(
                                dst, ps[:].rearrange("p (t b) -> p t b", b=16),
                                scal[:, m: m + 1])
                        else:
                            half = m - 4
                            dst = gi1_n[:].rearrange("p (t c) -> p t c", c=32)[
                                :, 8 * c:8 * (c + 1), half * 16:(half + 1) * 16]
                            nc.vector.tensor_scalar_add(
                                dst, ps[:].rearrange("p (t b) -> p t b", b=16),
                                scal[:, 4 + half: 5 + half])

                def gru_layer_step(t, layer):
                    if layer == 0:
                        whhT_l = whh0T
                        hprev16 = h0_16[:, 0:32] if t == 0 else out0_16[:, (t - 1) * 32: t * 32]
                        ho16 = out0_16
                        ppr, ppz, ppn = pp_r0, pp_z0, pp_n0
                        gin_rz, gin_n, bhn_b = gi0_rz, gi0_n, bhn0_b
                    else:
                        whhT_l = whh1T
                        hprev16 = h0_16[:, 32:64] if t == 0 else out1_16[:, (t - 1) * 32: t * 32]
                        ho16 = out1_16
                        ppr, ppz, ppn = pp_r1, pp_z1, pp_n1
                        gin_rz, gin_n, bhn_b = gi1_rz, gi1_n, bhn1_b

                    ps_r = ppr.tile([128, 32], F32, tag=f"r{layer}")
                    ps_z = ppz.tile([128, 32], F32, tag=f"z{layer}")
                    ps_n = ppn.tile([128, 32], F32, tag=f"n{layer}")
                    # r-gate first: its 4 matmuls + gi_r add + sigmoid_r form
                    # the critical chain; z-gate and n-path fill PE slack.
                    for mi in range(2):
                        for kk in (0, 1):
                            nc.tensor.matmul(
                                ps_r[:, mi * 16:(mi + 1) * 16],
                                whhT_l[:, kk * 768 + mi * 128: kk * 768 + (mi + 1) * 128],
                                hprev16[:, kk * 16:(kk + 1) * 16],
                                start=(mi == 0 and kk == 0), stop=False,
                                skip_group_check=True,
                            )
                    nc.tensor.matmul(ps_r[:], ident[:],
                                     gin_rz[:, t * 64:t * 64 + 32],
                                     start=False, stop=True,
                                     skip_group_check=True)
                    rz = gw.tile([128, 64], F32, tag=f"sig{layer}")
                    with tc.high_priority(offset=60):
                        nc.scalar.activation(rz[:, 0:32], ps_r[:], AF.Sigmoid)
                    for half in (0, 1):
                        mi = 4 + half
                        for kk in (0, 1):
                            nc.tensor.matmul(
                                ps_n[:, half * 16:(half + 1) * 16],
                                whhT_l[:, kk * 768 + mi * 128: kk * 768 + (mi + 1) * 128],
                                hprev16[:, kk * 16:(kk + 1) * 16],
                                start=(half == 0 and kk == 0), stop=False,
                                skip_group_check=True,
                            )
                    nc.tensor.matmul(ps_n[:, 0:32], ident[:], bhn_b[:],
                                     start=False, stop=True,
                                     skip_group_check=True)
                    for mi in range(2, 4):
                        for kk in (0, 1):
                            nc.tensor.matmul(
                                ps_z[:, (mi - 2) * 16:(mi - 1) * 16],
                                whhT_l[:, kk * 768 + mi * 128: kk * 768 + (mi + 1) * 128],
                                hprev16[:, kk * 16:(kk + 1) * 16],
                                start=(mi == 2 and kk == 0), stop=False,
                                skip_group_check=True,
                            )
                    nc.tensor.matmul(ps_z[:], ident[:],
                                     gin_rz[:, t * 64 + 32:(t + 1) * 64],
                                     start=False, stop=True,
                                     skip_group_check=True)
                    nc.scalar.activation(rz[:, 32:64], ps_z[:], AF.Sigmoid)
                    # z-branch (off the critical chain, on gpsimd):
                    # omz = 1 - z ; zh = z * h_prev
                    omz = gw.tile([128, 32], F32, tag=f"omz{layer}")
                    nc.vector.tensor_scalar(omz[:], rz[:, 32:64], -1.0, 1.0,
                                            op0=ALU.mult, op1=ALU.add)
                    zh = gw.tile([128, 32], F32, tag=f"zh{layer}")
                    nc.vector.tensor_mul(zh[:], rz[:, 32:64], hprev16[:])
                    # n-branch (critical chain)
                    with tc.high_priority(offset=60):
                        t1 = gw.tile([128, 32], F32, tag=f"t1_{layer}")
                        nc.vector.tensor_mul(t1[:], ps_n[:], rz[:, 0:32])
                        t2 = gw.tile([128, 32], F32, tag=f"t2_{layer}")
                        nc.vector.tensor_add(t2[:], t1[:], gin_n[:, t * 32:(t + 1) * 32])
                        n = gw.tile([128, 32], F32, tag=f"n_{layer}")
                        nc.scalar.activation(n[:], t2[:], AF.Tanh)
                        p = gw.tile([128, 32], F32, tag=f"p_{layer}")
                        nc.vector.tensor_mul(p[:], n[:], omz[:])
                        nc.vector.tensor_add(ho16[:, t * 32:(t + 1) * 32], p[:], zh[:])

                LAG = 9
                _l0only = os.environ.get("BASS_L0ONLY", "0") == "1"
                _nslots = (Z + LAG) if _phases == "full" else 0
                for slot in range(_nslots):
                    if slot < Z:
                        gru_layer_step(slot, 0)
                    if _l0only:
                        continue
                    if slot >= 8 and slot % 8 == 0 and (slot - 8) // 8 < Z // 8:
                        gi1_chunk((slot - 8) // 8)
                    if slot >= LAG:
                        u = slot - LAG
                        gru_layer_step(u, 1)
                        if (u + 1) % 16 == 0 and not _l0only:
                            blk = u // 16
                            nc.sync.dma_start(
                                d_out1[:, blk * 512:(blk + 1) * 512],
                                out1_16[:, blk * 512:(blk + 1) * 512],
                            )
                if _phases != "full" or _l0only:
                    nc.sync.dma_start(d_hn[:, 0:32], h0_16[:, 0:32])
                    nc.sync.dma_start(d_hn[:, 32:64], h0_16[:, 32:64])
                else:
                    if taps:
                        nc.sync.dma_start(d_dbgrz[:], gi0_rz[:])
                        nc.sync.dma_start(d_dbgn[:], gi0_n[:])
                        nc.sync.dma_start(d_dbgout0[:], out0_16[:])
                    nc.sync.dma_start(d_hn[:, 0:32], out0_16[:, (Z - 1) * 32: Z * 32])
                    nc.sync.dma_start(d_hn[:, 32:64], out1_16[:, (Z - 1) * 32: Z * 32])

    nc.compile()
    return nc


# ================= host side =================

def _prep_shared(inputs):
    g = lambda k: np.asarray(inputs[k], dtype=np.float32)
    f16 = np.float16
    out = {}
    out["whhT_f"] = np.ascontiguousarray(
        np.concatenate([g('t_Whh_f').T[:128], g('t_Whh_f').T[128:]], axis=1)).astype(f16)
    out["whhT_b"] = np.ascontiguousarray(
        np.concatenate([g('t_Whh_b').T[:128], g('t_Whh_b').T[128:]], axis=1)).astype(f16)
    out["wxb_f"] = np.stack([g('t_Wih_f')[:, 0], g('t_bih_f') + g('t_bhh_f')]).astype(f16)
    out["wxb_b"] = np.stack([g('t_Wih_b')[:, 0], g('t_bih_b') + g('t_bhh_b')]).astype(f16)
    fW = g('fus_W')[0]
    out["fusW"] = np.ascontiguousarray(
        fW.reshape(2, 2, 128).transpose(2, 1, 0).reshape(128, 4))
    # fusW[p, kk*2+d] = fW[d*256 + kk*128 + p]
    out["fusW"] = np.zeros((128, 4), np.float32)
    for kk in range(2):
        for d in range(2):
            out["fusW"][:, kk * 2 + d] = fW[d * 256 + kk * 128: d * 256 + (kk + 1) * 128]
    out["fusW"] = out["fusW"].astype(f16)
    out["fusb"] = np.full((128, 1), g('fus_b')[0], np.float32)

    out["wih0T"] = g('g_Wih0').T.astype(f16)

    def pack_T(wT):  # (256,768) -> (128,1536)
        return np.ascontiguousarray(
            np.concatenate([wT[:128], wT[128:]], axis=1))
    out["whh0T"] = pack_T(g('g_Whh0').T).astype(f16)
    out["whh1T"] = pack_T(g('g_Whh1').T).astype(f16)
    out["wih1T"] = pack_T(g('g_Wih1').T).astype(f16)

    def bcast_b(v):  # (256,) -> (128,32) packed, bcast over batch
        return np.repeat(v.reshape(2, 128).T[:, :, None], B, axis=2) \
            .reshape(128, 32).astype(f16)
    out["bhn0b"] = bcast_b(g('g_bhh0')[512:])
    out["bhn1b"] = bcast_b(g('g_bhh1')[512:])
    out["ident"] = np.eye(128, dtype=f16)
    out["scal"] = np.concatenate([
        (g('g_bih1') + g('g_bhh1'))[:512].reshape(4, 128).T,
        g('g_bih1')[512:].reshape(2, 128).T,
        (g('g_bih0') + g('g_bhh0'))[:512].reshape(4, 128).T,
        g('g_bih0')[512:].reshape(2, 128).T,
    ], axis=1).astype(np.float32)
    h0 = g('h0')
    out["h0p"] = np.ascontiguousarray(
        h0.reshape(2, B, 2, 128).transpose(3, 0, 2, 1).reshape(128, 64))
    return out


def _prep_core(x, i):
    xs = x[:, 16 * i:16 * (i + 1), :]
    u = xs[..., :M].transpose(2, 1, 0).reshape(M, 256)
    v = xs[..., M:].transpose(2, 1, 0).reshape(M, 256)
    xseqT = np.ascontiguousarray(np.concatenate([u, v], axis=1))
    xs100 = np.ones((2 * M, S), np.float32)
    xs100[0::2] = xseqT
    return {"xseqT": xseqT, "xs100": xs100.astype(np.float16)}


def kernel(**inputs):
    global _last_results
    if "nc" not in _nc_cache:
        _nc_cache["nc"] = build_nc()
    nc = _nc_cache["nc"]

    x = np.asarray(inputs["x"], dtype=np.float32)
    shared = _prep_shared(inputs)
    in_maps = []
    for i in range(NC):
        m = dict(shared)
        m.update(_prep_core(x, i))
        in_maps.append(m)

    # The axon-tunneled device occasionally sheds a transient
    # LoadExecutable/AwaitReady failure; retry a few times.
    import time as _time
    last_exc = None
    for attempt in range(4):
        try:
            res = run_bass_kernel_spmd(nc, in_maps, list(range(NC)))
            break
        except Exception as e:  # noqa: BLE001
            last_exc = e
            _time.sleep(20 * (attempt + 1))
    else:
        raise last_exc
    _last_results = res
    r0 = res.results[0]
    out1_d = r0["out1"]
    hn_d = r0["hn"]
    out1 = np.ascontiguousarray(
        out1_d.astype(np.float32).reshape(128, Z, 2, B)
        .transpose(3, 1, 2, 0).reshape(B, Z, H))
    hn = np.ascontiguousarray(
        hn_d.astype(np.float32).reshape(128, 2, 2, B)
        .transpose(1, 3, 2, 0).reshape(2, B, H))
    return out1, hn
